# revision 27
# baseline (speedup 1.0000x reference)
"""Trainium2 Bass kernel v3 for the GCN model (8 NeuronCores).

v3 redesign on top of v2.1 (trace-driven):
- The gather stream (SWDGE dma_gather, 4 queues) is the hard floor at
  ~2.3us per 1024-edge chunk; everything else is reorganized to run
  strictly under that rate so gathers never stall:
- Narrow-band segment matrices: edges are sorted by (run, blk, slot) so a
  128-edge tile spans <= ~24 dst slots.  The one-hot seg build compares
  against a 64-wide iota (dual 32-col bands) -> one [128,TPC,64] DVE
  is_equal per chunk (679ns vs 1212ns for the old full-width build), and
  the aggregation matmuls write 32/64-col column slices of the psum.
- Block accumulators live in PSUM across a superblock (SBW=14 blocks):
  stream order is half-major (w0,w1 = AG half A; w2,w3 = half B), and
  SB-major within a half.  The block psum is opened by a start=True
  matmul (self-loop inject from stash via identity rhs), accumulated by
  band MMs, closed once per half.  Half-A closes with a Scalar copy to a
  bf16 accA buffer; half-B re-injects accA (identity matmul), adds the
  rank-1 LayerNorm correction (-mu*wbar (x) sig row) inside PSUM via a
  1-partition matmul, and the epilogue reads PSUM directly.
  This deletes the SBUF f32 acc, all per-window psum->acc Vector adds,
  and the corr subtract; Vector keeps only seg builds + one dinv mult
  per block + stats adds.
- SBUF freed (acc 6.4MB, sigdr 3.2MB) funds accA (3.2MB bf16) and a
  deeper gather buffer pool (GBUF 20).
"""

import sys

sys.path.insert(0, "/opt/trn_rl_repo")

import numpy as np
import ml_dtypes

import concourse.bass as bass
import concourse.bacc as bacc
import concourse.mybir as mybir
import concourse.tile as tile
from concourse.bass_utils import run_bass_kernel_spmd
from concourse.library_config import mlp as gpsimd_mlp_lib
from concourse.masks import make_identity

NCORES = 8
N_NODES = 100_000
F = 128
NCLS = 10
LAYERS = 3
NGRAPH = 256
EPS = 1e-5
NSH = N_NODES // NCORES           # 12500
NBLK = 98
LASTW = NSH - (NBLK - 1) * 128    # 84
A_BLKS = 49
A_ROWS = A_BLKS * 128             # 6272
B_ROWS = NSH - A_ROWS             # 6228
REG_A = NCORES * A_ROWS           # 50176
REG_B = NCORES * B_ROWS           # 49824
WIN_BASE = [0, 4 * A_ROWS, REG_A, REG_A + 4 * B_ROWS]
WIN_SIZE = [4 * A_ROWS, 4 * A_ROWS, 4 * B_ROWS, 4 * B_ROWS]
NWIN = 4
SBW = 14                          # blocks per superblock
NSB = NBLK // SBW                 # 7
CHUNK = 1024
TPC = CHUNK // 128
SLABCH = 16
NQUEUE = 4
GBUF = 20
P0SLAB = 14

BF16 = mybir.dt.bfloat16
F32 = mybir.dt.float32
I16 = mybir.dt.int16
AOP = mybir.AluOpType
AF = mybir.ActivationFunctionType
BF = ml_dtypes.bfloat16


def _host_preprocess(edge_index):
    """v3 layout: edges sorted by (run, blk, slot) where
    run = (half, sb, win-within-half); regions sized per-(run,blk) to the
    max core count (SPMD-shared structure); runs padded to CHUNK."""
    src = np.asarray(edge_index[0], dtype=np.int64)
    dst = np.asarray(edge_index[1], dtype=np.int64)
    deg = np.bincount(dst, minlength=N_NODES).astype(np.float64) + 1.0
    dinv = (1.0 / np.sqrt(deg)).astype(np.float32)
    sig = (dinv.astype(np.float64) + np.bincount(
        dst, weights=dinv[src].astype(np.float64), minlength=N_NODES)
    ).astype(np.float32)

    # source -> (window, in-window idx) under the permuted hws_full layout
    c_s = src // NSH
    r = src % NSH
    rhalf = (r >= A_ROWS).astype(np.int64)
    win = rhalf * 2 + (c_s // 4)
    grow = np.where(rhalf == 0,
                    c_s * A_ROWS + r,
                    REG_A + c_s * B_ROWS + (r - A_ROWS))
    widx = grow - np.asarray(WIN_BASE, dtype=np.int64)[win]

    core = dst // NSH
    blk = (dst % NSH) // 128
    slot = (dst % NSH) % 128

    half = win // 2
    sb = blk // SBW
    runid = (half * NSB + sb) * 2 + (win % 2)     # [0, 28)
    NRUN = 2 * NSB * 2

    key = (core * NRUN + runid) * NBLK + blk
    order = np.argsort(key * 128 + (core * 0 + slot), kind="stable")
    # (sort by key then slot; key*128+slot fits int64)
    key_s = key[order]
    widx_s = widx[order]
    slot_s = slot[order]
    core_s = core[order]

    ngroups = NCORES * NRUN * NBLK
    counts = np.bincount(key_s, minlength=ngroups).reshape(NCORES, NRUN, NBLK)
    starts = np.zeros(ngroups + 1, dtype=np.int64)
    np.cumsum(counts.reshape(-1), out=starts[1:])

    # shared region sizes: max over cores per (run, blk); zero where the
    # run's sb doesn't own the block
    reg_sz = counts.max(axis=0)                    # [NRUN, NBLK]
    for rn in range(NRUN):
        sbr = (rn // 2) % NSB
        for b in range(NBLK):
            if b // SBW != sbr:
                assert reg_sz[rn, b] == 0
    # run lengths padded to CHUNK
    run_len = reg_sz.sum(axis=1)
    run_pad = (-run_len) % CHUNK
    run_plen = run_len + run_pad
    run_off = np.zeros(NRUN + 1, dtype=np.int64)
    np.cumsum(run_plen, out=run_off[1:])
    ecap = int(run_off[-1])
    assert ecap % CHUNK == 0
    nchunk = ecap // CHUNK
    TT = ecap // 128

    # region start offset per (run, blk)
    reg_off = np.zeros((NRUN, NBLK), dtype=np.int64)
    for rn in range(NRUN):
        pos = run_off[rn]
        sbr = (rn // 2) % NSB
        for b in range(sbr * SBW, min((sbr + 1) * SBW, NBLK)):
            reg_off[rn, b] = pos
            pos += reg_sz[rn, b]

    # chunk -> window (chunks are run-pure)
    chunk_win = np.empty(nchunk, dtype=np.int64)
    for rn in range(NRUN):
        c0, c1 = run_off[rn] // CHUNK, run_off[rn + 1] // CHUNK
        h, w2 = rn // (2 * NSB), rn % 2
        chunk_win[c0:c1] = h * 2 + w2

    # ---- per-tile parts (shared across cores) ----
    # part = (blk, lo, o, W): seg cols [lo, lo+W) -> psum cols [o, o+W)
    # per-core slot extremes per (tile, blk) to pick shared bands
    tile_parts = [[] for _ in range(TT)]
    # build per (run, blk): positions [reg_off, reg_off+reg_sz)
    # slot min/max per tile-part across cores:
    smin = np.full((TT, 2), 300, dtype=np.int64)   # up to 2 parts/tile
    smax = np.full((TT, 2), -1, dtype=np.int64)
    part_blk = np.full((TT, 2), -1, dtype=np.int64)
    for rn in range(NRUN):
        sbr = (rn // 2) % NSB
        for b in range(sbr * SBW, min((sbr + 1) * SBW, NBLK)):
            p0, p1 = reg_off[rn, b], reg_off[rn, b] + reg_sz[rn, b]
            if p1 == p0:
                continue
            t0, t1 = p0 // 128, (p1 - 1) // 128
            for t in range(t0, t1 + 1):
                # which part index within tile t is this block?
                parts = tile_parts[t]
                pi = len(parts)
                assert pi < 2, f"3-part tile {t}"
                parts.append(b)
                part_blk[t, pi] = b
    # per-core per-tile-part slot ranges
    for c in range(NCORES):
        for rn in range(NRUN):
            sbr = (rn // 2) % NSB
            for b in range(sbr * SBW, min((sbr + 1) * SBW, NBLK)):
                g = (c * NRUN + rn) * NBLK + b
                s0, s1 = starts[g], starts[g + 1]
                n = s1 - s0
                if n == 0:
                    continue
                p0 = reg_off[rn, b]
                sl = slot_s[s0:s1]
                # positions p0 .. p0+n-1
                t0, t1 = p0 // 128, (p0 + n - 1) // 128
                for t in range(t0, t1 + 1):
                    lo = max(p0, t * 128) - p0
                    hi = min(p0 + n, (t + 1) * 128) - p0
                    pi = list(part_blk[t]).index(b)
                    smin[t, pi] = min(smin[t, pi], int(sl[lo:hi].min()))
                    smax[t, pi] = max(smax[t, pi], int(sl[lo:hi].max()))

    # choose bands
    band = np.zeros((TT, 2, 3), dtype=np.int64)    # (o, lo, W)
    for t in range(TT):
        nparts = len(tile_parts[t])
        for pi in range(nparts):
            b = part_blk[t, pi]
            w = 128 if b < NBLK - 1 else LASTW
            span = smax[t, pi] - smin[t, pi] + 1
            if nparts == 1:
                W = 32 if span <= 32 else 64
                assert span <= 64, f"span {span} tile {t}"
                lo = 0
            else:
                W = 32
                assert span <= 32, f"2-part span {span} tile {t}"
                lo = pi * 32
            o = min(int(smin[t, pi]), max(0, w - W))
            band[t, pi] = (o, lo, W)

    # ---- fill idx / encoded slots per core ----
    # Pad positions must NOT all point at row 0: hundreds of same-address
    # descriptors serialize on one HBM bank and stall the whole DMA queue
    # at run boundaries.  Spread pad/default indices across the window.
    idx16 = np.zeros((NCORES, ecap), dtype=np.int16)
    for rn in range(NRUN):
        o0 = int(run_off[rn]) + int(run_len[rn])
        o1 = int(run_off[rn + 1])
        if o1 > o0:
            wsz = WIN_SIZE[(rn // (2 * NSB)) * 2 + rn % 2]
            spread = ((np.arange(o1 - o0, dtype=np.int64) * 97) % wsz
                      ).astype(np.int16)
            idx16[:, o0:o1] = spread[None, :]
    slots = np.full((NCORES, ecap), 300, dtype=np.float32)
    for c in range(NCORES):
        for rn in range(NRUN):
            sbr = (rn // 2) % NSB
            for b in range(sbr * SBW, min((sbr + 1) * SBW, NBLK)):
                g = (c * NRUN + rn) * NBLK + b
                s0, s1 = starts[g], starts[g + 1]
                n = s1 - s0
                if n == 0:
                    continue
                p0 = reg_off[rn, b]
                idx16[c, p0:p0 + n] = widx_s[s0:s1].astype(np.int16)
                # region shortfall pads: recycle this core's real source
                # rows (finite, spread) instead of hammering row 0
                short = int(reg_sz[rn, b]) - n
                if short > 0:
                    rep = np.resize(widx_s[s0:s1], short).astype(np.int16)
                    idx16[c, p0 + n:p0 + n + short] = rep
                # encode slots per covering tile part
                sl = slot_s[s0:s1].astype(np.float32)
                enc = np.empty(n, dtype=np.float32)
                t0, t1 = p0 // 128, (p0 + n - 1) // 128
                for t in range(t0, t1 + 1):
                    lo = max(p0, t * 128) - p0
                    hi = min(p0 + n, (t + 1) * 128) - p0
                    pi = list(part_blk[t]).index(b)
                    o, blo, W = band[t, pi]
                    enc[lo:hi] = sl[lo:hi] - o + blo
                    assert (enc[lo:hi] >= blo).all() and \
                           (enc[lo:hi] < blo + W).all()
                slots[c, p0:p0 + n] = enc

    idxw = idx16.reshape(NCORES, -1, 16).transpose(0, 2, 1)
    idxw = np.ascontiguousarray(np.tile(idxw, (1, 8, 1)))
    slotw = np.ascontiguousarray(
        slots.reshape(NCORES, TT, 128).transpose(0, 2, 1)).astype(BF)

    # ---- emission script helpers ----
    # last real part per (half, blk): (tile, part_idx)
    last_part = {}
    first_alpha_chunk = np.zeros(NSB, dtype=np.int64)
    first_beta_chunk = np.zeros(NSB, dtype=np.int64)
    for h in range(2):
        for b in range(NBLK):
            sbr = b // SBW
            # runs for this half/sb: rn0 (w even), rn1 (w odd)
            rn1 = (h * NSB + sbr) * 2 + 1
            rn0 = rn1 - 1
            use = rn1 if reg_sz[rn1, b] > 0 else rn0
            p1 = reg_off[use, b] + reg_sz[use, b]
            t = (p1 - 1) // 128
            pi = list(part_blk[t]).index(b)
            last_part[(h, b)] = (int(t), int(pi))
    for sbr in range(NSB):
        first_alpha_chunk[sbr] = run_off[(0 * NSB + sbr) * 2] // CHUNK
        first_beta_chunk[sbr] = run_off[(1 * NSB + sbr) * 2] // CHUNK

    meta = dict(TT=TT, nchunk=nchunk, chunk_win=chunk_win,
                tile_parts=tile_parts, part_blk=part_blk, band=band,
                last_part=last_part,
                first_alpha_chunk=first_alpha_chunk,
                first_beta_chunk=first_beta_chunk)
    return dinv, sig, idxw, slotw, meta


def _build_program(meta):
    TT = meta["TT"]
    NCHUNK = meta["nchunk"]
    chunk_win = meta["chunk_win"]
    tile_parts = meta["tile_parts"]
    part_blk = meta["part_blk"]
    band = meta["band"]
    last_part = meta["last_part"]
    first_alpha_chunk = meta["first_alpha_chunk"]
    first_beta_chunk = meta["first_beta_chunk"]
    ECAP = TT * 128

    nc = bacc.Bacc("TRN2", target_bir_lowering=False, debug=False,
                   num_devices=NCORES, num_swdge_queues=NQUEUE)

    xT_in = nc.declare_dram_parameter("xT", [F, NBLK * 128], BF16, isOutput=False)
    idx_in = nc.declare_dram_parameter("idx", [128, ECAP // 16], I16, isOutput=False)
    slot_in = nc.declare_dram_parameter("slot", [128, TT], BF16, isOutput=False)
    dinvrep_in = nc.declare_dram_parameter("dinvrep", [128, NBLK * 128], BF16, isOutput=False)
    sigrow_in = nc.declare_dram_parameter("sigrow", [1, NBLK * 128], BF16, isOutput=False)
    dinvw_in = nc.declare_dram_parameter("dinvw", [128, NBLK], F32, isOutput=False)
    pslot_in = nc.declare_dram_parameter("pslot", [128, NBLK], BF16, isOutput=False)
    iota64_in = nc.declare_dram_parameter("iota64", [128, TPC * 64], BF16, isOutput=False)
    iota256_in = nc.declare_dram_parameter("iota256", [128, 256], BF16, isOutput=False)
    lin1W_in = nc.declare_dram_parameter("lin1W", [F, F], BF16, isOutput=False)
    lin1b_in = nc.declare_dram_parameter("lin1b", [F, 1], F32, isOutput=False)
    convW_in = nc.declare_dram_parameter("convW", [F, LAYERS * F], BF16, isOutput=False)
    convb_in = nc.declare_dram_parameter("convb", [F, LAYERS], F32, isOutput=False)
    wbarnr_in = nc.declare_dram_parameter("wbarnr", [1, (LAYERS - 1) * F], F32, isOutput=False)
    mlpW1_in = nc.declare_dram_parameter("mlpW1", [F, F], BF16, isOutput=False)
    mlpb1_in = nc.declare_dram_parameter("mlpb1", [F, 1], F32, isOutput=False)
    mlpW2_in = nc.declare_dram_parameter("mlpW2", [F, NCLS], BF16, isOutput=False)
    mlpb2r_in = nc.declare_dram_parameter("mlpb2r", [128, NCLS], F32, isOutput=False)
    invcntr_in = nc.declare_dram_parameter("invcntr", [128, NGRAPH], F32, isOutput=False)
    out_ext = nc.declare_dram_parameter("out", [NGRAPH, NCLS], F32, isOutput=True)

    rg = [list(range(NCORES))]

    with tile.TileContext(nc) as tc:
        with tc.tile_pool(name="const", bufs=1) as cst, \
             tc.tile_pool(name="big", bufs=1) as big, \
             tc.tile_pool(name="work", bufs=4) as work, \
             tc.tile_pool(name="segp", bufs=4) as segpool, \
             tc.tile_pool(name="gbuf", bufs=GBUF) as gpool, \
             tc.tile_pool(name="idxs", bufs=3) as ipool, \
             tc.tile_pool(name="agg", bufs=4, space="PSUM") as aggp, \
             tc.tile_pool(name="mmp", bufs=3, space="PSUM") as mmp, \
             tc.tile_pool(name="ppool", bufs=1, space="PSUM") as ppool, \
             tc.tile_pool(name="dram2", bufs=1, space="DRAM") as dram2, \
             tc.tile_pool(name="dram", bufs=1, space="DRAM") as dram:

            nc.gpsimd.load_library(gpsimd_mlp_lib)
            nidx_reg = nc.gpsimd.to_reg(CHUNK)

            lin1W = cst.tile([F, F], BF16)
            nc.sync.dma_start(out=lin1W[:], in_=lin1W_in[:])
            lin1b = cst.tile([F, 1], F32)
            nc.sync.dma_start(out=lin1b[:], in_=lin1b_in[:])
            convW = cst.tile([F, LAYERS * F], BF16)
            nc.sync.dma_start(out=convW[:], in_=convW_in[:])
            convb = cst.tile([F, LAYERS], F32)
            nc.sync.dma_start(out=convb[:], in_=convb_in[:])
            dinvw = cst.tile([128, NBLK], F32)
            nc.sync.dma_start(out=dinvw[:], in_=dinvw_in[:])
            slot_t = cst.tile([128, TT], BF16)
            nc.sync.dma_start(out=slot_t[:], in_=slot_in[:])
            iota64 = cst.tile([128, TPC * 64], BF16)
            nc.sync.dma_start(out=iota64[:], in_=iota64_in[:])
            dinvrep = cst.tile([128, NBLK * 128], BF16)
            sigrow = cst.tile([1, NBLK * 128], BF16)
            pslot = cst.tile([128, NBLK], BF16)
            iota256 = cst.tile([128, 256], BF16)
            wbarnr = cst.tile([1, (LAYERS - 1) * F], F32)
            invcntr = cst.tile([128, NGRAPH], F32)
            mlpW1 = cst.tile([F, F], BF16)
            mlpb1 = cst.tile([F, 1], F32)
            mlpW2 = cst.tile([F, NCLS], BF16)
            mlpb2r = cst.tile([128, NCLS], F32)
            ones_col = cst.tile([128, 1], BF16)
            nc.vector.memset(ones_col[:], 1.0)
            ones_row1 = cst.tile([1, 128], BF16)
            nc.vector.memset(ones_row1[:], 1.0)
            ident = cst.tile([128, 128], BF16)
            make_identity(nc, ident[:])

            stash = big.tile([128, NBLK * 128], BF16)
            accA = big.tile([128, NBLK * 128], BF16)


            hwsA = dram.tile([A_ROWS, F], BF16)
            hwsB = dram.tile([B_ROWS, F], BF16)
            hwsfA = [dram.tile([REG_A, F], BF16, addr_space="Shared",
                                name=f"hwsfA{l}") for l in range(LAYERS)]
            hwsfB = [dram.tile([REG_B, F], BF16, addr_space="Shared",
                               name=f"hwsfB{l}") for l in range(LAYERS)]

            def win_ap_l(li, w):
                if w < 2:
                    return hwsfA[li][w * (4 * A_ROWS):(w + 1) * (4 * A_ROWS), :]
                return hwsfB[li][(w - 2) * (4 * B_ROWS):
                                 (w - 1) * (4 * B_ROWS), :]

            def bw(b):
                return 128 if b < NBLK - 1 else LASTW

            def shard_write(b, eng=None):
                w = bw(b)
                src = stash[:w, b * 128:(b + 1) * 128]
                e = eng if eng is not None else nc.sync
                if b < A_BLKS:
                    e.dma_start(out=hwsA[b * 128:b * 128 + w, :], in_=src)
                else:
                    o = (b - A_BLKS) * 128
                    e.dma_start(out=hwsB[o:o + w, :], in_=src)

            def emit_ag(which, idx):
                if which == 0:
                    nc.gpsimd.collective_compute(
                        "AllGather", AOP.bypass, replica_groups=rg,
                        ins=[hwsA[:]], outs=[hwsfA[idx][:]])
                else:
                    nc.gpsimd.collective_compute(
                        "AllGather", AOP.bypass, replica_groups=rg,
                        ins=[hwsB[:]], outs=[hwsfB[idx][:]])

            # ---------------- P0 ----------
            pending_ag = [False]
            p0_ranges = []
            for lo, hi in ((0, A_BLKS), (A_BLKS, NBLK)):
                s = lo
                while s < hi:
                    p0_ranges.append((s, min(P0SLAB, hi - s)))
                    s += min(P0SLAB, hi - s)
            for (s0, ns) in p0_ranges:
                xsl = work.tile([128, P0SLAB * 128], BF16, tag="xsl")
                nc.sync.dma_start(
                    out=xsl[:, :ns * 128],
                    in_=xT_in[:, s0 * 128:(s0 + ns) * 128])
                j = 0
                while j < ns:
                    nj = min(4, ns - j)
                    ps = mmp.tile([128, 512], F32, tag="mm")
                    nc.tensor.matmul(out=ps[:, :nj * 128], lhsT=lin1W[:],
                                     rhs=xsl[:, j * 128:(j + nj) * 128],
                                     start=True, stop=True)
                    h1b = work.tile([128, 512], BF16, tag="h1")
                    nc.scalar.activation(out=h1b[:, :nj * 128],
                                         in_=ps[:, :nj * 128], func=AF.Relu,
                                         bias=lin1b[:], scale=1.0)
                    for k in range(nj):
                        b = s0 + j + k
                        w = bw(b)
                        ps2 = mmp.tile([128, F], F32, tag="mm")
                        nc.tensor.matmul(out=ps2[:w, :],
                                         lhsT=h1b[:, k * 128:k * 128 + w],
                                         rhs=convW[:, 0:F],
                                         start=True, stop=True)
                        nc.scalar.activation(
                            out=stash[:w, b * 128:(b + 1) * 128],
                            in_=ps2[:w, :],
                            func=AF.Copy, bias=0.0, scale=dinvw[:w, b:b + 1])
                        shard_write(b, eng=nc.scalar)
                        if b == A_BLKS - 1:
                            emit_ag(0, 0)
                    j += nj
            pending_ag[0] = True

            # deferred const loads (first use: stats chain / beta epilogues)
            nc.sync.dma_start(out=dinvrep[:], in_=dinvrep_in[:])
            nc.sync.dma_start(out=sigrow[:], in_=sigrow_in[:])
            nc.sync.dma_start(out=pslot[:], in_=pslot_in[:])
            nc.sync.dma_start(out=iota256[:], in_=iota256_in[:])
            nc.sync.dma_start(out=wbarnr[:], in_=wbarnr_in[:])
            nc.sync.dma_start(out=invcntr[:], in_=invcntr_in[:])
            nc.sync.dma_start(out=mlpW1[:], in_=mlpW1_in[:])
            nc.sync.dma_start(out=mlpb1[:], in_=mlpb1_in[:])
            nc.sync.dma_start(out=mlpW2[:], in_=mlpW2_in[:])
            nc.sync.dma_start(out=mlpb2r[:], in_=mlpb2r_in[:])

            # ---------------- conv layers ----------------
            pool_ps = None
            pending_chain = None
            layer_ctx = {"musd": None, "wmrow": None}
            for li in range(LAYERS):
                last = li == LAYERS - 1
                stats = work.tile([128, 2], F32, tag="stats")
                nc.vector.memset(stats[:], 0.0)
                if last:
                    pool_ps = ppool.tile([128, NGRAPH], F32, tag="pool")
                nepi = [0]

                aggt = {}       # blk -> (bank psum tile, col offset)

                def agg_ap(b):
                    t, co = aggt[b]
                    return t[:, co:co + bw(b)]

                def alloc_sb(sbr, _li=li):
                    nb = min((sbr + 1) * SBW, NBLK) - sbr * SBW
                    tiles = []
                    for k in range((nb + 3) // 4):
                        bank = aggp.tile([128, 512], F32, tag="agg",
                                         name=f"agg_{_li}_{sbr}_{k}")
                        tiles.append(bank)
                    for ib in range(nb):
                        aggt[sbr * SBW + ib] = (tiles[ib // 4], (ib % 4) * 128)

                def inject_alpha(sbr, _li=li):
                    alloc_sb(sbr)
                    started = set()
                    for b in range(sbr * SBW, min((sbr + 1) * SBW, NBLK)):
                        w = bw(b)
                        bank = id(aggt[b][0])
                        st = bank not in started
                        started.add(bank)
                        nc.tensor.matmul(
                            out=agg_ap(b), lhsT=stash[:w, b * 128:(b + 1) * 128],
                            rhs=ident[:w, :w], start=st, stop=False,
                            skip_group_check=True)

                def inject_beta(sbr, _li=li):
                    alloc_sb(sbr)
                    started = set()
                    for b in range(sbr * SBW, min((sbr + 1) * SBW, NBLK)):
                        w = bw(b)
                        bank = id(aggt[b][0])
                        st = bank not in started
                        started.add(bank)
                        nc.tensor.matmul(
                            out=agg_ap(b), lhsT=ident[:],
                            rhs=accA[:, b * 128:b * 128 + w],
                            start=st, stop=False, skip_group_check=True)

                def close_alpha(b):
                    w = bw(b)
                    nc.scalar.activation(
                        out=accA[:, b * 128:b * 128 + w], in_=agg_ap(b),
                        func=AF.Copy, bias=0.0, scale=1.0)
                    del aggt[b]

                def emit_epilogue(b, _li=li, _last=last, _stats=stats,
                                  _ctx=layer_ctx):
                    nonlocal pool_ps
                    w = bw(b)
                    if _li > 0:
                        nc.tensor.matmul(
                            out=agg_ap(b), lhsT=_ctx["wmrow"][:],
                            rhs=sigrow[:, b * 128:b * 128 + w],
                            start=False, stop=True, skip_group_check=True)
                    tmp32 = work.tile([128, 128], F32, tag="tmp32")
                    nc.vector.tensor_tensor(
                        out=tmp32[:, :w], in0=agg_ap(b),
                        in1=dinvrep[:, b * 128:b * 128 + w], op=AOP.mult)
                    hb16 = work.tile([128, 128], BF16, tag="hb16")
                    s1 = work.tile([128, 1], F32, tag="s1")
                    nc.scalar.activation(
                        out=hb16[:, :w], in_=tmp32[:, :w], func=AF.Relu,
                        bias=convb[:, _li:_li + 1],
                        scale=(_ctx["musd"][:, 3:4] if _li > 0 else 1.0),
                        accum_out=s1[:])
                    sq = work.tile([128, 128], F32, tag="sq")
                    s2 = work.tile([128, 1], F32, tag="s2")
                    nc.scalar.activation(out=sq[:, :w], in_=hb16[:, :w],
                                         func=AF.Square, bias=0.0, scale=1.0,
                                         accum_out=s2[:])
                    nc.vector.tensor_tensor(out=_stats[:, 0:1], in0=_stats[:, 0:1],
                                            in1=s1[:], op=AOP.add)
                    nc.vector.tensor_tensor(out=_stats[:, 1:2], in0=_stats[:, 1:2],
                                            in1=s2[:], op=AOP.add)
                    del aggt[b]
                    if not _last:
                        psP = mmp.tile([128, F], F32, tag="mm")
                        nc.tensor.matmul(
                            out=psP[:w, :], lhsT=hb16[:, :w],
                            rhs=convW[:, (_li + 1) * F:(_li + 2) * F],
                            start=True, stop=True)
                        nc.scalar.activation(
                            out=stash[:w, b * 128:(b + 1) * 128],
                            in_=psP[:w, :], func=AF.Copy, bias=0.0,
                            scale=dinvw[:w, b:b + 1])
                        shard_write(b)
                        if b == A_BLKS - 1:
                            emit_ag(0, _li + 1)
                        elif b == NBLK - 1:
                            pending_ag[0] = True
                    else:
                        ps_t = mmp.tile([128, 128], BF16, tag="mm")
                        nc.tensor.transpose(out=ps_t[:], in_=hb16[:],
                                            identity=ident[:])
                        h3 = work.tile([128, 128], BF16, tag="h3")
                        nc.scalar.activation(out=h3[:w, :], in_=ps_t[:w, :],
                                             func=AF.Copy, bias=0.0, scale=1.0)
                        segg = work.tile([128, NGRAPH], BF16, tag="segg")
                        nc.vector.tensor_tensor(
                            out=segg[:w, :],
                            in0=pslot[:w, b:b + 1].to_broadcast([w, NGRAPH]),
                            in1=iota256[:w, :], op=AOP.is_equal)
                        nc.tensor.matmul(out=pool_ps[:], lhsT=h3[:w, :],
                                         rhs=segg[:w, :],
                                         start=(nepi[0] == 0),
                                         stop=(nepi[0] == NBLK - 1),
                                         skip_group_check=True)
                    nepi[0] += 1

                # ---- gather + aggregate stream ----
                idx_slab = None
                idx_slabs = {}
                # map: after tile t in half h -> blocks to finish
                fin_after = {}
                for (h, b), (t, pi) in last_part.items():
                    fin_after.setdefault((h, t), []).append(b)

                for ch in range(NCHUNK):
                    wq = int(chunk_win[ch])
                    h = wq // 2
                    if ch == 96 and pending_ag[0]:
                        emit_ag(1, li)
                        pending_ag[0] = False
                    if ch == 48 and pending_chain is not None:
                        pending_chain()
                        pending_chain = None
                    # SB injects
                    for sbr in range(NSB):
                        if first_alpha_chunk[sbr] == ch:
                            inject_alpha(sbr)
                        if first_beta_chunk[sbr] == ch:
                            inject_beta(sbr)
                    if ch % SLABCH == 0:
                        if ch == 0:
                            idx_slabs[0] = ipool.tile(
                                [128, SLABCH * CHUNK // 16], I16, tag="idxslab",
                                name=f"islab_{li}_0")
                            nc.sync.dma_start(
                                out=idx_slabs[0][:],
                                in_=idx_in[:, 0:SLABCH * CHUNK // 16])
                        idx_slab = idx_slabs[ch // SLABCH]
                        # prefetch next slab now (sync engine, runs ahead)
                        nxt = ch // SLABCH + 1
                        if nxt * SLABCH < NCHUNK:
                            idx_slabs[nxt] = ipool.tile(
                                [128, SLABCH * CHUNK // 16], I16, tag="idxslab",
                                name=f"islab_{li}_{nxt}")
                            wsl = min(SLABCH * CHUNK,
                                      ECAP - nxt * SLABCH * CHUNK) // 16
                            nc.sync.dma_start(
                                out=idx_slabs[nxt][:, :wsl],
                                in_=idx_in[:, nxt * SLABCH * CHUNK // 16:
                                           nxt * SLABCH * CHUNK // 16 + wsl])
                    gb = gpool.tile([128, TPC, F], BF16, tag="gb")
                    off = (ch % SLABCH) * (CHUNK // 16)
                    nc.gpsimd.dma_gather(
                        gb[:], win_ap_l(li, wq),
                        idx_slab[:, off:off + CHUNK // 16],
                        CHUNK, nidx_reg, F, single_packet=True,
                        queue_num=ch % NQUEUE)
                    gt0 = ch * TPC
                    any_part = any(tile_parts[gt0 + t] for t in range(TPC))
                    seg = None
                    if any_part:
                        seg = segpool.tile([128, TPC, 64], BF16, tag="seg")
                        nc.vector.tensor_tensor(
                            out=seg[:],
                            in0=iota64[:].rearrange("p (a b) -> p a b", a=TPC),
                            in1=slot_t[:, gt0:gt0 + TPC].unsqueeze(2)
                                .to_broadcast([128, TPC, 64]),
                            op=AOP.is_equal)
                    for t in range(TPC):
                        gt = gt0 + t
                        parts = tile_parts[gt]
                        for pi in range(len(parts)):
                            b = part_blk[gt, pi]
                            o, blo, W = band[gt, pi]
                            is_last = last_part.get((h, b)) == (gt, pi)
                            stopf = is_last and (h == 0 or li == 0)
                            bt, co = aggt[b]
                            nc.tensor.matmul(
                                out=bt[:, co + o:co + o + W],
                                lhsT=gb[:, t, :],
                                rhs=seg[:, t, blo:blo + W],
                                start=False, stop=stopf,
                                skip_group_check=True)
                        for b in fin_after.get((h, gt), ()):
                            if h == 0:
                                close_alpha(b)
                            else:
                                emit_epilogue(b)

                # ---- stats AllReduce chain (deferred into next layer) ----
                def emit_stats_chain(_stats=stats, _ctx=layer_ctx, _li=li,
                                     _last=last):
                    st_in = dram2.tile([128, 2], F32, tag="stin")
                    st_out = dram2.tile([128, 2], F32, tag="stout")
                    nc.sync.dma_start(out=st_in[:], in_=_stats[:])
                    nc.gpsimd.collective_compute(
                        "AllReduce", AOP.add, replica_groups=rg,
                        ins=[st_in[:]], outs=[st_out[:]])
                    stg = work.tile([128, 2], F32, tag="stg")
                    nc.sync.dma_start(out=stg[:], in_=st_out[:])
                    stg16 = work.tile([128, 2], BF16, tag="stg16")
                    nc.vector.tensor_copy(out=stg16[:], in_=stg[:])
                    ps_s = mmp.tile([1, 2], F32, tag="mm")
                    nc.tensor.matmul(out=ps_s[:], lhsT=ones_col[:], rhs=stg16[:],
                                     start=True, stop=True)
                    sc = work.tile([1, 4], F32, tag="sc")
                    nc.scalar.activation(out=sc[:, 0:2], in_=ps_s[:], func=AF.Copy,
                                         bias=0.0, scale=1.0 / (N_NODES * F))
                    nc.vector.tensor_tensor(out=sc[:, 2:3], in0=sc[:, 0:1],
                                            in1=sc[:, 0:1], op=AOP.mult)
                    nc.vector.tensor_tensor(out=sc[:, 2:3], in0=sc[:, 1:2],
                                            in1=sc[:, 2:3], op=AOP.subtract)
                    nc.vector.tensor_scalar(out=sc[:, 2:3], in0=sc[:, 2:3],
                                            scalar1=EPS, scalar2=None, op0=AOP.add)
                    nc.vector.reciprocal(out=sc[:, 3:4], in_=sc[:, 2:3])
                    nc.scalar.activation(out=sc[:, 3:4], in_=sc[:, 3:4],
                                         func=AF.Sqrt, bias=0.0, scale=1.0)
                    sc16 = work.tile([1, 4], BF16, tag="sc16")
                    nc.vector.tensor_copy(out=sc16[:], in_=sc[:])
                    ps_b = mmp.tile([128, 4], F32, tag="mm")
                    nc.tensor.matmul(out=ps_b[:], lhsT=ones_row1[:], rhs=sc16[:],
                                     start=True, stop=True)
                    musd = work.tile([128, 4], F32, tag="musd")
                    nc.vector.tensor_copy(out=musd[:], in_=ps_b[:])
                    _ctx["musd"] = musd
                    if not _last:
                        wmrow = work.tile([1, 128], BF16, tag="wmrow")
                        nc.vector.tensor_scalar(
                            out=wmrow[:],
                            in0=wbarnr[:, _li * F:(_li + 1) * F],
                            scalar1=musd[0:1, 0:1], scalar2=None,
                            op0=AOP.mult)
                        _ctx["wmrow"] = wmrow

                if last:
                    emit_stats_chain()
                else:
                    pending_chain = emit_stats_chain

            # ---------------- pool affine + MLP head ----------------
            pooledT = work.tile([128, NGRAPH], F32, tag="pooledT")
            nc.scalar.activation(out=pooledT[:], in_=pool_ps[:],
                                 func=AF.Copy, bias=0.0, scale=1.0)
            pl_in = dram2.tile([128, NGRAPH], F32, tag="plin")
            pl_out = dram2.tile([128, NGRAPH], F32, tag="plout")
            nc.sync.dma_start(out=pl_in[:], in_=pooledT[:])
            nc.gpsimd.collective_compute(
                "AllReduce", AOP.add, replica_groups=rg,
                ins=[pl_in[:]], outs=[pl_out[:]])
            pooled = work.tile([128, NGRAPH], F32, tag="pooled2")
            nc.sync.dma_start(out=pooled[:], in_=pl_out[:])
            nc.vector.tensor_tensor(out=pooled[:], in0=pooled[:],
                                    in1=invcntr[:], op=AOP.mult)
            mrs = work.tile([128, 1], F32, tag="mrs")
            nc.vector.tensor_tensor(out=mrs[:], in0=layer_ctx["musd"][:, 0:1],
                                    in1=layer_ctx["musd"][:, 3:4], op=AOP.mult)
            nc.vector.tensor_scalar(out=pooled[:], in0=pooled[:],
                                    scalar1=layer_ctx["musd"][:, 3:4],
                                    scalar2=mrs[:],
                                    op0=AOP.mult, op1=AOP.subtract)
            pooled16 = work.tile([128, NGRAPH], BF16, tag="pooled16")
            nc.vector.tensor_copy(out=pooled16[:], in_=pooled[:])

            ps_g = mmp.tile([128, NGRAPH], F32, tag="mm")
            nc.tensor.matmul(out=ps_g[:], lhsT=mlpW1[:], rhs=pooled16[:],
                             start=True, stop=True)
            gT = work.tile([128, NGRAPH], BF16, tag="gT")
            nc.scalar.activation(out=gT[:], in_=ps_g[:], func=AF.Relu,
                                 bias=mlpb1[:], scale=1.0)
            for halfi in range(2):
                ps_sc = mmp.tile([128, NCLS], F32, tag="mm")
                nc.tensor.matmul(out=ps_sc[:],
                                 lhsT=gT[:, halfi * 128:(halfi + 1) * 128],
                                 rhs=mlpW2[:], start=True, stop=True)
                scr = work.tile([128, NCLS], F32, tag="scr")
                nc.vector.tensor_tensor(out=scr[:], in0=ps_sc[:],
                                        in1=mlpb2r[:], op=AOP.add)
                mx = work.tile([128, 1], F32, tag="mx")
                nc.vector.tensor_reduce(out=mx[:], in_=scr[:],
                                        axis=mybir.AxisListType.X,
                                        op=AOP.max)
                nc.vector.tensor_scalar(out=scr[:], in0=scr[:], scalar1=mx[:],
                                        scalar2=None, op0=AOP.subtract)
                ex = work.tile([128, NCLS], F32, tag="ex")
                sm = work.tile([128, 1], F32, tag="sm")
                nc.scalar.activation(out=ex[:], in_=scr[:], func=AF.Exp,
                                     bias=0.0, scale=1.0, accum_out=sm[:])
                ls = work.tile([128, 1], F32, tag="ls")
                nc.scalar.activation(out=ls[:], in_=sm[:], func=AF.Ln,
                                     bias=0.0, scale=1.0)
                nc.vector.tensor_scalar(out=scr[:], in0=scr[:], scalar1=ls[:],
                                        scalar2=None, op0=AOP.subtract)
                nc.sync.dma_start(
                    out=out_ext[halfi * 128:(halfi + 1) * 128, :],
                    in_=scr[:])

    nc.compile()
    return nc


def _wrap_cols(vec, fill):
    padded = np.full(NBLK * 128, fill, np.float32)
    padded[:NSH] = vec
    return np.ascontiguousarray(padded.reshape(NBLK, 128).T)


def _prepare(inputs):
    x = np.asarray(inputs["x"], dtype=np.float32)
    edge_index = np.asarray(inputs["edge_index"])
    batch = np.asarray(inputs["batch"], dtype=np.int64)

    dinv, sig, idxw, slotw, meta = _host_preprocess(edge_index)

    cnt = np.bincount(batch, minlength=NGRAPH).astype(np.float64)
    invcnt = (1.0 / np.maximum(cnt, 1.0)).astype(np.float32)
    iota64 = np.tile(np.arange(64, dtype=np.float32), (128, TPC))
    iota256 = np.broadcast_to(np.arange(256, dtype=np.float32), (128, 256))

    lin1_W = np.asarray(inputs["lin1_W"], np.float32)
    lin1_b = np.asarray(inputs["lin1_b"], np.float32)
    conv_W = np.asarray(inputs["conv_W"], np.float32)
    conv_b = np.asarray(inputs["conv_b"], np.float32)
    mlp_W1 = np.asarray(inputs["mlp_W1"], np.float32)
    mlp_b1 = np.asarray(inputs["mlp_b1"], np.float32)
    mlp_W2 = np.asarray(inputs["mlp_W2"], np.float32)
    mlp_b2 = np.asarray(inputs["mlp_b2"], np.float32)

    convW_cat = np.concatenate([conv_W[l] for l in range(LAYERS)], axis=1)
    wbarnr = -np.concatenate(
        [conv_W[l].sum(axis=0) for l in range(1, LAYERS)])[None, :]

    in_maps = []
    for c in range(NCORES):
        lo, hi = c * NSH, (c + 1) * NSH
        xT = np.zeros((F, NBLK * 128), np.float32)
        xT[:, :NSH] = x[lo:hi].T
        dinv_pad = np.zeros(NBLK * 128, np.float32)
        dinv_pad[:NSH] = dinv[lo:hi]
        sig_pad = np.zeros(NBLK * 128, np.float32)
        sig_pad[:NSH] = sig[lo:hi]
        in_maps.append({
            "xT": xT.astype(BF),
            "idx": idxw[c],
            "slot": slotw[c],
            "dinvrep": np.ascontiguousarray(
                np.broadcast_to(dinv_pad, (128, NBLK * 128))).astype(BF),
            "sigrow": np.ascontiguousarray(sig_pad[None, :]).astype(BF),
            "dinvw": _wrap_cols(dinv[lo:hi], 0.0),
            "pslot": _wrap_cols(batch[lo:hi].astype(np.float32),
                                300.0).astype(BF),
            "iota64": iota64.astype(BF),
            "iota256": iota256.astype(BF),
            "lin1W": lin1_W.astype(BF),
            "lin1b": np.ascontiguousarray(lin1_b.reshape(F, 1)),
            "convW": convW_cat.astype(BF),
            "convb": np.ascontiguousarray(conv_b.T),
            "wbarnr": np.ascontiguousarray(wbarnr.astype(np.float32)),
            "mlpW1": mlp_W1.astype(BF),
            "mlpb1": np.ascontiguousarray(mlp_b1.reshape(F, 1)),
            "mlpW2": mlp_W2.astype(BF),
            "mlpb2r": np.ascontiguousarray(
                np.broadcast_to(mlp_b2, (128, NCLS)).astype(np.float32)),
            "invcntr": np.ascontiguousarray(
                np.broadcast_to(invcnt, (128, NGRAPH))),
        })
    return meta, in_maps


_CACHED = {}


def kernel_run(inputs, trace=False):
    meta, in_maps = _prepare(inputs)
    key = meta["TT"]
    if key not in _CACHED:
        _CACHED[key] = _build_program(meta)
    nc = _CACHED[key]
    res = run_bass_kernel_spmd(nc, in_maps, core_ids=list(range(NCORES)),
                               trace=trace)
    out = np.asarray(res.results[0]["out"], dtype=np.float32)
    return out, res.exec_time_ns


def kernel(**inputs):
    out, _ = kernel_run(inputs, trace=False)
    return out


# revision 28
# speedup vs baseline: 1.0083x; 1.0083x over previous
"""Trainium2 Bass kernel v3 for the GCN model (8 NeuronCores).

v3 redesign on top of v2.1 (trace-driven):
- The gather stream (SWDGE dma_gather, 4 queues) is the hard floor at
  ~2.3us per 1024-edge chunk; everything else is reorganized to run
  strictly under that rate so gathers never stall:
- Narrow-band segment matrices: edges are sorted by (run, blk, slot) so a
  128-edge tile spans <= ~24 dst slots.  The one-hot seg build compares
  against a 64-wide iota (dual 32-col bands) -> one [128,TPC,64] DVE
  is_equal per chunk (679ns vs 1212ns for the old full-width build), and
  the aggregation matmuls write 32/64-col column slices of the psum.
- Block accumulators live in PSUM across a superblock (SBW=14 blocks):
  stream order is half-major (w0,w1 = AG half A; w2,w3 = half B), and
  SB-major within a half.  The block psum is opened by a start=True
  matmul (self-loop inject from stash via identity rhs), accumulated by
  band MMs, closed once per half.  Half-A closes with a Scalar copy to a
  bf16 accA buffer; half-B re-injects accA (identity matmul), adds the
  rank-1 LayerNorm correction (-mu*wbar (x) sig row) inside PSUM via a
  1-partition matmul, and the epilogue reads PSUM directly.
  This deletes the SBUF f32 acc, all per-window psum->acc Vector adds,
  and the corr subtract; Vector keeps only seg builds + one dinv mult
  per block + stats adds.
- SBUF freed (acc 6.4MB, sigdr 3.2MB) funds accA (3.2MB bf16) and a
  deeper gather buffer pool (GBUF 20).
"""

import sys

sys.path.insert(0, "/opt/trn_rl_repo")

import numpy as np
import ml_dtypes

import concourse.bass as bass
import concourse.bacc as bacc
import concourse.mybir as mybir
import concourse.tile as tile
from concourse.bass_utils import run_bass_kernel_spmd
from concourse.library_config import mlp as gpsimd_mlp_lib
from concourse.masks import make_identity

NCORES = 8
N_NODES = 100_000
F = 128
NCLS = 10
LAYERS = 3
NGRAPH = 256
EPS = 1e-5
NSH = N_NODES // NCORES           # 12500
NBLK = 98
LASTW = NSH - (NBLK - 1) * 128    # 84
A_BLKS = 49
A_ROWS = A_BLKS * 128             # 6272
B_ROWS = NSH - A_ROWS             # 6228
REG_A = NCORES * A_ROWS           # 50176
REG_B = NCORES * B_ROWS           # 49824
WIN_BASE = [0, 4 * A_ROWS, REG_A, REG_A + 4 * B_ROWS]
WIN_SIZE = [4 * A_ROWS, 4 * A_ROWS, 4 * B_ROWS, 4 * B_ROWS]
NWIN = 4
SBW = 14                          # blocks per superblock
NSB = NBLK // SBW                 # 7
CHUNK = 1024
TPC = CHUNK // 128
SLABCH = 16
NQUEUE = 4
GBUF = 20
P0SLAB = 14

BF16 = mybir.dt.bfloat16
F32 = mybir.dt.float32
I16 = mybir.dt.int16
AOP = mybir.AluOpType
AF = mybir.ActivationFunctionType
BF = ml_dtypes.bfloat16


def _host_preprocess(edge_index):
    """v3 layout: edges sorted by (run, blk, slot) where
    run = (half, sb, win-within-half); regions sized per-(run,blk) to the
    max core count (SPMD-shared structure); runs padded to CHUNK."""
    src = np.asarray(edge_index[0], dtype=np.int64)
    dst = np.asarray(edge_index[1], dtype=np.int64)
    deg = np.bincount(dst, minlength=N_NODES).astype(np.float64) + 1.0
    dinv = (1.0 / np.sqrt(deg)).astype(np.float32)
    sig = (dinv.astype(np.float64) + np.bincount(
        dst, weights=dinv[src].astype(np.float64), minlength=N_NODES)
    ).astype(np.float32)

    # source -> (window, in-window idx) under the permuted hws_full layout
    c_s = src // NSH
    r = src % NSH
    rhalf = (r >= A_ROWS).astype(np.int64)
    win = rhalf * 2 + (c_s // 4)
    grow = np.where(rhalf == 0,
                    c_s * A_ROWS + r,
                    REG_A + c_s * B_ROWS + (r - A_ROWS))
    widx = grow - np.asarray(WIN_BASE, dtype=np.int64)[win]

    core = dst // NSH
    blk = (dst % NSH) // 128
    slot = (dst % NSH) % 128

    half = win // 2
    sb = blk // SBW
    runid = (half * NSB + sb) * 2 + (win % 2)     # [0, 28)
    NRUN = 2 * NSB * 2

    key = (core * NRUN + runid) * NBLK + blk
    order = np.argsort(key * 128 + (core * 0 + slot), kind="stable")
    # (sort by key then slot; key*128+slot fits int64)
    key_s = key[order]
    widx_s = widx[order]
    slot_s = slot[order]
    core_s = core[order]

    ngroups = NCORES * NRUN * NBLK
    counts = np.bincount(key_s, minlength=ngroups).reshape(NCORES, NRUN, NBLK)
    starts = np.zeros(ngroups + 1, dtype=np.int64)
    np.cumsum(counts.reshape(-1), out=starts[1:])

    # shared region sizes: max over cores per (run, blk); zero where the
    # run's sb doesn't own the block
    reg_sz = counts.max(axis=0)                    # [NRUN, NBLK]
    for rn in range(NRUN):
        sbr = (rn // 2) % NSB
        for b in range(NBLK):
            if b // SBW != sbr:
                assert reg_sz[rn, b] == 0
    # run lengths padded to CHUNK
    run_len = reg_sz.sum(axis=1)
    run_pad = (-run_len) % CHUNK
    run_plen = run_len + run_pad
    run_off = np.zeros(NRUN + 1, dtype=np.int64)
    np.cumsum(run_plen, out=run_off[1:])
    ecap = int(run_off[-1])
    assert ecap % CHUNK == 0
    nchunk = ecap // CHUNK
    TT = ecap // 128

    # region start offset per (run, blk)
    reg_off = np.zeros((NRUN, NBLK), dtype=np.int64)
    for rn in range(NRUN):
        pos = run_off[rn]
        sbr = (rn // 2) % NSB
        for b in range(sbr * SBW, min((sbr + 1) * SBW, NBLK)):
            reg_off[rn, b] = pos
            pos += reg_sz[rn, b]

    # chunk -> window (chunks are run-pure)
    chunk_win = np.empty(nchunk, dtype=np.int64)
    for rn in range(NRUN):
        c0, c1 = run_off[rn] // CHUNK, run_off[rn + 1] // CHUNK
        h, w2 = rn // (2 * NSB), rn % 2
        chunk_win[c0:c1] = h * 2 + w2

    # ---- per-tile parts (shared across cores) ----
    # part = (blk, lo, o, W): seg cols [lo, lo+W) -> psum cols [o, o+W)
    # per-core slot extremes per (tile, blk) to pick shared bands
    tile_parts = [[] for _ in range(TT)]
    # build per (run, blk): positions [reg_off, reg_off+reg_sz)
    # slot min/max per tile-part across cores:
    smin = np.full((TT, 2), 300, dtype=np.int64)   # up to 2 parts/tile
    smax = np.full((TT, 2), -1, dtype=np.int64)
    part_blk = np.full((TT, 2), -1, dtype=np.int64)
    for rn in range(NRUN):
        sbr = (rn // 2) % NSB
        for b in range(sbr * SBW, min((sbr + 1) * SBW, NBLK)):
            p0, p1 = reg_off[rn, b], reg_off[rn, b] + reg_sz[rn, b]
            if p1 == p0:
                continue
            t0, t1 = p0 // 128, (p1 - 1) // 128
            for t in range(t0, t1 + 1):
                # which part index within tile t is this block?
                parts = tile_parts[t]
                pi = len(parts)
                assert pi < 2, f"3-part tile {t}"
                parts.append(b)
                part_blk[t, pi] = b
    # per-core per-tile-part slot ranges
    for c in range(NCORES):
        for rn in range(NRUN):
            sbr = (rn // 2) % NSB
            for b in range(sbr * SBW, min((sbr + 1) * SBW, NBLK)):
                g = (c * NRUN + rn) * NBLK + b
                s0, s1 = starts[g], starts[g + 1]
                n = s1 - s0
                if n == 0:
                    continue
                p0 = reg_off[rn, b]
                sl = slot_s[s0:s1]
                # positions p0 .. p0+n-1
                t0, t1 = p0 // 128, (p0 + n - 1) // 128
                for t in range(t0, t1 + 1):
                    lo = max(p0, t * 128) - p0
                    hi = min(p0 + n, (t + 1) * 128) - p0
                    pi = list(part_blk[t]).index(b)
                    smin[t, pi] = min(smin[t, pi], int(sl[lo:hi].min()))
                    smax[t, pi] = max(smax[t, pi], int(sl[lo:hi].max()))

    # choose bands
    band = np.zeros((TT, 2, 3), dtype=np.int64)    # (o, lo, W)
    for t in range(TT):
        nparts = len(tile_parts[t])
        for pi in range(nparts):
            b = part_blk[t, pi]
            w = 128 if b < NBLK - 1 else LASTW
            span = smax[t, pi] - smin[t, pi] + 1
            if nparts == 1:
                W = 32 if span <= 32 else 64
                assert span <= 64, f"span {span} tile {t}"
                lo = 0
            else:
                W = 32
                assert span <= 32, f"2-part span {span} tile {t}"
                lo = pi * 32
            o = min(int(smin[t, pi]), max(0, w - W))
            band[t, pi] = (o, lo, W)

    # ---- fill idx / encoded slots per core ----
    # Pad positions must NOT all point at row 0: hundreds of same-address
    # descriptors serialize on one HBM bank and stall the whole DMA queue
    # at run boundaries.  Spread pad/default indices across the window.
    idx16 = np.zeros((NCORES, ecap), dtype=np.int16)
    for rn in range(NRUN):
        o0 = int(run_off[rn]) + int(run_len[rn])
        o1 = int(run_off[rn + 1])
        if o1 > o0:
            wsz = WIN_SIZE[(rn // (2 * NSB)) * 2 + rn % 2]
            spread = ((np.arange(o1 - o0, dtype=np.int64) * 97) % wsz
                      ).astype(np.int16)
            idx16[:, o0:o1] = spread[None, :]
    slots = np.full((NCORES, ecap), 300, dtype=np.float32)
    for c in range(NCORES):
        for rn in range(NRUN):
            sbr = (rn // 2) % NSB
            for b in range(sbr * SBW, min((sbr + 1) * SBW, NBLK)):
                g = (c * NRUN + rn) * NBLK + b
                s0, s1 = starts[g], starts[g + 1]
                n = s1 - s0
                if n == 0:
                    continue
                p0 = reg_off[rn, b]
                idx16[c, p0:p0 + n] = widx_s[s0:s1].astype(np.int16)
                # region shortfall pads: recycle this core's real source
                # rows (finite, spread) instead of hammering row 0
                short = int(reg_sz[rn, b]) - n
                if short > 0:
                    rep = np.resize(widx_s[s0:s1], short).astype(np.int16)
                    idx16[c, p0 + n:p0 + n + short] = rep
                # encode slots per covering tile part
                sl = slot_s[s0:s1].astype(np.float32)
                enc = np.empty(n, dtype=np.float32)
                t0, t1 = p0 // 128, (p0 + n - 1) // 128
                for t in range(t0, t1 + 1):
                    lo = max(p0, t * 128) - p0
                    hi = min(p0 + n, (t + 1) * 128) - p0
                    pi = list(part_blk[t]).index(b)
                    o, blo, W = band[t, pi]
                    enc[lo:hi] = sl[lo:hi] - o + blo
                    assert (enc[lo:hi] >= blo).all() and \
                           (enc[lo:hi] < blo + W).all()
                slots[c, p0:p0 + n] = enc

    idxw = idx16.reshape(NCORES, -1, 16).transpose(0, 2, 1)
    idxw = np.ascontiguousarray(np.tile(idxw, (1, 8, 1)))
    slotw = np.ascontiguousarray(
        slots.reshape(NCORES, TT, 128).transpose(0, 2, 1)).astype(BF)

    # ---- emission script helpers ----
    # last real part per (half, blk): (tile, part_idx)
    last_part = {}
    first_alpha_chunk = np.zeros(NSB, dtype=np.int64)
    first_beta_chunk = np.zeros(NSB, dtype=np.int64)
    for h in range(2):
        for b in range(NBLK):
            sbr = b // SBW
            # runs for this half/sb: rn0 (w even), rn1 (w odd)
            rn1 = (h * NSB + sbr) * 2 + 1
            rn0 = rn1 - 1
            use = rn1 if reg_sz[rn1, b] > 0 else rn0
            p1 = reg_off[use, b] + reg_sz[use, b]
            t = (p1 - 1) // 128
            pi = list(part_blk[t]).index(b)
            last_part[(h, b)] = (int(t), int(pi))
    for sbr in range(NSB):
        first_alpha_chunk[sbr] = run_off[(0 * NSB + sbr) * 2] // CHUNK
        first_beta_chunk[sbr] = run_off[(1 * NSB + sbr) * 2] // CHUNK

    meta = dict(TT=TT, nchunk=nchunk, chunk_win=chunk_win,
                tile_parts=tile_parts, part_blk=part_blk, band=band,
                last_part=last_part,
                first_alpha_chunk=first_alpha_chunk,
                first_beta_chunk=first_beta_chunk)
    return dinv, sig, idxw, slotw, meta


def _build_program(meta):
    TT = meta["TT"]
    NCHUNK = meta["nchunk"]
    chunk_win = meta["chunk_win"]
    tile_parts = meta["tile_parts"]
    part_blk = meta["part_blk"]
    band = meta["band"]
    last_part = meta["last_part"]
    first_alpha_chunk = meta["first_alpha_chunk"]
    first_beta_chunk = meta["first_beta_chunk"]
    ECAP = TT * 128

    nc = bacc.Bacc("TRN2", target_bir_lowering=False, debug=False,
                   num_devices=NCORES, num_swdge_queues=NQUEUE)

    xT_in = nc.declare_dram_parameter("xT", [F, NBLK * 128], BF16, isOutput=False)
    idx_in = nc.declare_dram_parameter("idx", [128, ECAP // 16], I16, isOutput=False)
    slot_in = nc.declare_dram_parameter("slot", [128, TT], BF16, isOutput=False)
    dinvrep_in = nc.declare_dram_parameter("dinvrep", [128, NBLK * 128], BF16, isOutput=False)
    sigrow_in = nc.declare_dram_parameter("sigrow", [1, NBLK * 128], BF16, isOutput=False)
    dinvw_in = nc.declare_dram_parameter("dinvw", [128, NBLK], F32, isOutput=False)
    pslot_in = nc.declare_dram_parameter("pslot", [128, NBLK], BF16, isOutput=False)
    iota64_in = nc.declare_dram_parameter("iota64", [128, TPC * 64], BF16, isOutput=False)
    iota256_in = nc.declare_dram_parameter("iota256", [128, 256], BF16, isOutput=False)
    lin1W_in = nc.declare_dram_parameter("lin1W", [F, F], BF16, isOutput=False)
    lin1b_in = nc.declare_dram_parameter("lin1b", [F, 1], F32, isOutput=False)
    convW_in = nc.declare_dram_parameter("convW", [F, LAYERS * F], BF16, isOutput=False)
    convb_in = nc.declare_dram_parameter("convb", [F, LAYERS], F32, isOutput=False)
    wbarnr_in = nc.declare_dram_parameter("wbarnr", [1, (LAYERS - 1) * F], F32, isOutput=False)
    mlpW1_in = nc.declare_dram_parameter("mlpW1", [F, F], BF16, isOutput=False)
    mlpb1_in = nc.declare_dram_parameter("mlpb1", [F, 1], F32, isOutput=False)
    mlpW2_in = nc.declare_dram_parameter("mlpW2", [F, NCLS], BF16, isOutput=False)
    mlpb2r_in = nc.declare_dram_parameter("mlpb2r", [128, NCLS], F32, isOutput=False)
    invcntr_in = nc.declare_dram_parameter("invcntr", [128, NGRAPH], F32, isOutput=False)
    out_ext = nc.declare_dram_parameter("out", [NGRAPH, NCLS], F32, isOutput=True)

    rg = [list(range(NCORES))]

    with tile.TileContext(nc) as tc:
        with tc.tile_pool(name="const", bufs=1) as cst, \
             tc.tile_pool(name="big", bufs=1) as big, \
             tc.tile_pool(name="work", bufs=4) as work, \
             tc.tile_pool(name="segp", bufs=4) as segpool, \
             tc.tile_pool(name="gbuf", bufs=GBUF) as gpool, \
             tc.tile_pool(name="idxs", bufs=3) as ipool, \
             tc.tile_pool(name="agg", bufs=4, space="PSUM") as aggp, \
             tc.tile_pool(name="mmp", bufs=3, space="PSUM") as mmp, \
             tc.tile_pool(name="ppool", bufs=1, space="PSUM") as ppool, \
             tc.tile_pool(name="dram2", bufs=1, space="DRAM") as dram2, \
             tc.tile_pool(name="dram", bufs=1, space="DRAM") as dram:

            nc.gpsimd.load_library(gpsimd_mlp_lib)
            nidx_reg = nc.gpsimd.to_reg(CHUNK)

            lin1W = cst.tile([F, F], BF16)
            nc.sync.dma_start(out=lin1W[:], in_=lin1W_in[:])
            lin1b = cst.tile([F, 1], F32)
            nc.sync.dma_start(out=lin1b[:], in_=lin1b_in[:])
            convW = cst.tile([F, LAYERS * F], BF16)
            nc.sync.dma_start(out=convW[:], in_=convW_in[:])
            convb = cst.tile([F, LAYERS], F32)
            nc.sync.dma_start(out=convb[:], in_=convb_in[:])
            dinvw = cst.tile([128, NBLK], F32)
            nc.sync.dma_start(out=dinvw[:], in_=dinvw_in[:])
            slot_t = cst.tile([128, TT], BF16)
            nc.sync.dma_start(out=slot_t[:], in_=slot_in[:])
            iota64 = cst.tile([128, TPC * 64], BF16)
            nc.sync.dma_start(out=iota64[:], in_=iota64_in[:])
            dinvrep = cst.tile([128, NBLK * 128], BF16)
            sigrow = cst.tile([1, NBLK * 128], BF16)
            pslot = cst.tile([128, NBLK], BF16)
            iota256 = cst.tile([128, 256], BF16)
            wbarnr = cst.tile([1, (LAYERS - 1) * F], F32)
            invcntr = cst.tile([128, NGRAPH], F32)
            mlpW1 = cst.tile([F, F], BF16)
            mlpb1 = cst.tile([F, 1], F32)
            mlpW2 = cst.tile([F, NCLS], BF16)
            mlpb2r = cst.tile([128, NCLS], F32)
            ones_col = cst.tile([128, 1], BF16)
            nc.vector.memset(ones_col[:], 1.0)
            ones_row1 = cst.tile([1, 128], BF16)
            nc.vector.memset(ones_row1[:], 1.0)
            ident = cst.tile([128, 128], BF16)
            make_identity(nc, ident[:])

            stash = big.tile([128, NBLK * 128], BF16)
            accA = big.tile([128, NBLK * 128], BF16)


            hwsA = dram.tile([A_ROWS, F], BF16)
            hwsB = dram.tile([B_ROWS, F], BF16)
            hwsfA = [dram.tile([REG_A, F], BF16, addr_space="Shared",
                                name=f"hwsfA{l}") for l in range(LAYERS)]
            hwsfB = [dram.tile([REG_B, F], BF16, addr_space="Shared",
                               name=f"hwsfB{l}") for l in range(LAYERS)]

            def win_ap_l(li, w):
                if w < 2:
                    return hwsfA[li][w * (4 * A_ROWS):(w + 1) * (4 * A_ROWS), :]
                return hwsfB[li][(w - 2) * (4 * B_ROWS):
                                 (w - 1) * (4 * B_ROWS), :]

            def bw(b):
                return 128 if b < NBLK - 1 else LASTW

            def shard_write(b):
                w = bw(b)
                src = stash[:w, b * 128:(b + 1) * 128]
                if b < A_BLKS:
                    nc.sync.dma_start(out=hwsA[b * 128:b * 128 + w, :], in_=src)
                else:
                    o = (b - A_BLKS) * 128
                    nc.sync.dma_start(out=hwsB[o:o + w, :], in_=src)

            def emit_ag(which, idx):
                if which == 0:
                    nc.gpsimd.collective_compute(
                        "AllGather", AOP.bypass, replica_groups=rg,
                        ins=[hwsA[:]], outs=[hwsfA[idx][:]])
                else:
                    nc.gpsimd.collective_compute(
                        "AllGather", AOP.bypass, replica_groups=rg,
                        ins=[hwsB[:]], outs=[hwsfB[idx][:]])

            # ---------------- P0 ----------
            pending_ag = [False]
            p0_ranges = []
            for lo, hi in ((0, A_BLKS), (A_BLKS, NBLK)):
                s = lo
                while s < hi:
                    p0_ranges.append((s, min(P0SLAB, hi - s)))
                    s += min(P0SLAB, hi - s)
            for (s0, ns) in p0_ranges:
                xsl = work.tile([128, P0SLAB * 128], BF16, tag="xsl")
                nc.sync.dma_start(
                    out=xsl[:, :ns * 128],
                    in_=xT_in[:, s0 * 128:(s0 + ns) * 128])
                j = 0
                while j < ns:
                    nj = min(4, ns - j)
                    ps = mmp.tile([128, 512], F32, tag="mm")
                    nc.tensor.matmul(out=ps[:, :nj * 128], lhsT=lin1W[:],
                                     rhs=xsl[:, j * 128:(j + nj) * 128],
                                     start=True, stop=True)
                    h1b = work.tile([128, 512], BF16, tag="h1")
                    nc.scalar.activation(out=h1b[:, :nj * 128],
                                         in_=ps[:, :nj * 128], func=AF.Relu,
                                         bias=lin1b[:], scale=1.0)
                    for k in range(nj):
                        b = s0 + j + k
                        w = bw(b)
                        ps2 = mmp.tile([128, F], F32, tag="mm")
                        nc.tensor.matmul(out=ps2[:w, :],
                                         lhsT=h1b[:, k * 128:k * 128 + w],
                                         rhs=convW[:, 0:F],
                                         start=True, stop=True)
                        nc.scalar.activation(
                            out=stash[:w, b * 128:(b + 1) * 128],
                            in_=ps2[:w, :],
                            func=AF.Copy, bias=0.0, scale=dinvw[:w, b:b + 1])
                        shard_write(b)
                        if b == A_BLKS - 1:
                            emit_ag(0, 0)
                    j += nj
            pending_ag[0] = True

            # deferred const loads (first use: stats chain / beta epilogues)
            nc.sync.dma_start(out=dinvrep[:], in_=dinvrep_in[:])
            nc.sync.dma_start(out=sigrow[:], in_=sigrow_in[:])
            nc.sync.dma_start(out=pslot[:], in_=pslot_in[:])
            nc.sync.dma_start(out=iota256[:], in_=iota256_in[:])
            nc.sync.dma_start(out=wbarnr[:], in_=wbarnr_in[:])
            nc.sync.dma_start(out=invcntr[:], in_=invcntr_in[:])
            nc.sync.dma_start(out=mlpW1[:], in_=mlpW1_in[:])
            nc.sync.dma_start(out=mlpb1[:], in_=mlpb1_in[:])
            nc.sync.dma_start(out=mlpW2[:], in_=mlpW2_in[:])
            nc.sync.dma_start(out=mlpb2r[:], in_=mlpb2r_in[:])

            # ---------------- conv layers ----------------
            pool_ps = None
            pending_chain = None
            layer_ctx = {"musd": None, "wmrow": None}
            for li in range(LAYERS):
                last = li == LAYERS - 1
                stats = work.tile([128, 2], F32, tag="stats")
                nc.vector.memset(stats[:], 0.0)
                if last:
                    pool_ps = ppool.tile([128, NGRAPH], F32, tag="pool")
                nepi = [0]

                aggt = {}       # blk -> (bank psum tile, col offset)

                def agg_ap(b):
                    t, co = aggt[b]
                    return t[:, co:co + bw(b)]

                def alloc_sb(sbr, _li=li):
                    nb = min((sbr + 1) * SBW, NBLK) - sbr * SBW
                    tiles = []
                    for k in range((nb + 3) // 4):
                        bank = aggp.tile([128, 512], F32, tag="agg",
                                         name=f"agg_{_li}_{sbr}_{k}")
                        tiles.append(bank)
                    for ib in range(nb):
                        aggt[sbr * SBW + ib] = (tiles[ib // 4], (ib % 4) * 128)

                def inject_alpha(sbr, _li=li):
                    alloc_sb(sbr)
                    started = set()
                    for b in range(sbr * SBW, min((sbr + 1) * SBW, NBLK)):
                        w = bw(b)
                        bank = id(aggt[b][0])
                        st = bank not in started
                        started.add(bank)
                        nc.tensor.matmul(
                            out=agg_ap(b), lhsT=stash[:w, b * 128:(b + 1) * 128],
                            rhs=ident[:w, :w], start=st, stop=False,
                            skip_group_check=True)

                def inject_beta(sbr, _li=li):
                    alloc_sb(sbr)
                    started = set()
                    for b in range(sbr * SBW, min((sbr + 1) * SBW, NBLK)):
                        w = bw(b)
                        bank = id(aggt[b][0])
                        st = bank not in started
                        started.add(bank)
                        nc.tensor.matmul(
                            out=agg_ap(b), lhsT=ident[:],
                            rhs=accA[:, b * 128:b * 128 + w],
                            start=st, stop=False, skip_group_check=True)

                def close_alpha(b):
                    w = bw(b)
                    nc.scalar.activation(
                        out=accA[:, b * 128:b * 128 + w], in_=agg_ap(b),
                        func=AF.Copy, bias=0.0, scale=1.0)
                    del aggt[b]

                def emit_epilogue(b, _li=li, _last=last, _stats=stats,
                                  _ctx=layer_ctx):
                    nonlocal pool_ps
                    w = bw(b)
                    if _li > 0:
                        nc.tensor.matmul(
                            out=agg_ap(b), lhsT=_ctx["wmrow"][:],
                            rhs=sigrow[:, b * 128:b * 128 + w],
                            start=False, stop=True, skip_group_check=True)
                    tmp32 = work.tile([128, 128], F32, tag="tmp32")
                    nc.vector.tensor_tensor(
                        out=tmp32[:, :w], in0=agg_ap(b),
                        in1=dinvrep[:, b * 128:b * 128 + w], op=AOP.mult)
                    hb16 = work.tile([128, 128], BF16, tag="hb16")
                    s1 = work.tile([128, 1], F32, tag="s1")
                    nc.scalar.activation(
                        out=hb16[:, :w], in_=tmp32[:, :w], func=AF.Relu,
                        bias=convb[:, _li:_li + 1],
                        scale=(_ctx["musd"][:, 3:4] if _li > 0 else 1.0),
                        accum_out=s1[:])
                    sq = work.tile([128, 128], F32, tag="sq")
                    s2 = work.tile([128, 1], F32, tag="s2")
                    nc.scalar.activation(out=sq[:, :w], in_=hb16[:, :w],
                                         func=AF.Square, bias=0.0, scale=1.0,
                                         accum_out=s2[:])
                    nc.vector.tensor_tensor(out=_stats[:, 0:1], in0=_stats[:, 0:1],
                                            in1=s1[:], op=AOP.add)
                    nc.vector.tensor_tensor(out=_stats[:, 1:2], in0=_stats[:, 1:2],
                                            in1=s2[:], op=AOP.add)
                    del aggt[b]
                    if not _last:
                        psP = mmp.tile([128, F], F32, tag="mm")
                        nc.tensor.matmul(
                            out=psP[:w, :], lhsT=hb16[:, :w],
                            rhs=convW[:, (_li + 1) * F:(_li + 2) * F],
                            start=True, stop=True)
                        nc.scalar.activation(
                            out=stash[:w, b * 128:(b + 1) * 128],
                            in_=psP[:w, :], func=AF.Copy, bias=0.0,
                            scale=dinvw[:w, b:b + 1])
                        shard_write(b)
                        if b == A_BLKS - 1:
                            emit_ag(0, _li + 1)
                        elif b == NBLK - 1:
                            pending_ag[0] = True
                    else:
                        ps_t = mmp.tile([128, 128], BF16, tag="mm")
                        nc.tensor.transpose(out=ps_t[:], in_=hb16[:],
                                            identity=ident[:])
                        h3 = work.tile([128, 128], BF16, tag="h3")
                        nc.scalar.activation(out=h3[:w, :], in_=ps_t[:w, :],
                                             func=AF.Copy, bias=0.0, scale=1.0)
                        segg = work.tile([128, NGRAPH], BF16, tag="segg")
                        nc.vector.tensor_tensor(
                            out=segg[:w, :],
                            in0=pslot[:w, b:b + 1].to_broadcast([w, NGRAPH]),
                            in1=iota256[:w, :], op=AOP.is_equal)
                        nc.tensor.matmul(out=pool_ps[:], lhsT=h3[:w, :],
                                         rhs=segg[:w, :],
                                         start=(nepi[0] == 0),
                                         stop=(nepi[0] == NBLK - 1),
                                         skip_group_check=True)
                    nepi[0] += 1

                # ---- gather + aggregate stream ----
                idx_slab = None
                idx_slabs = {}
                # map: after tile t in half h -> blocks to finish
                fin_after = {}
                for (h, b), (t, pi) in last_part.items():
                    fin_after.setdefault((h, t), []).append(b)

                for ch in range(NCHUNK):
                    wq = int(chunk_win[ch])
                    h = wq // 2
                    if ch == 96 and pending_ag[0]:
                        emit_ag(1, li)
                        pending_ag[0] = False
                    if ch == 48 and pending_chain is not None:
                        pending_chain()
                        pending_chain = None
                    # SB injects
                    for sbr in range(NSB):
                        if first_alpha_chunk[sbr] == ch:
                            inject_alpha(sbr)
                        if first_beta_chunk[sbr] == ch:
                            inject_beta(sbr)
                    if ch % SLABCH == 0:
                        if ch == 0:
                            idx_slabs[0] = ipool.tile(
                                [128, SLABCH * CHUNK // 16], I16, tag="idxslab",
                                name=f"islab_{li}_0")
                            nc.sync.dma_start(
                                out=idx_slabs[0][:],
                                in_=idx_in[:, 0:SLABCH * CHUNK // 16])
                        idx_slab = idx_slabs[ch // SLABCH]
                        # prefetch next slab now (sync engine, runs ahead)
                        nxt = ch // SLABCH + 1
                        if nxt * SLABCH < NCHUNK:
                            idx_slabs[nxt] = ipool.tile(
                                [128, SLABCH * CHUNK // 16], I16, tag="idxslab",
                                name=f"islab_{li}_{nxt}")
                            wsl = min(SLABCH * CHUNK,
                                      ECAP - nxt * SLABCH * CHUNK) // 16
                            nc.sync.dma_start(
                                out=idx_slabs[nxt][:, :wsl],
                                in_=idx_in[:, nxt * SLABCH * CHUNK // 16:
                                           nxt * SLABCH * CHUNK // 16 + wsl])
                    gb = gpool.tile([128, TPC, F], BF16, tag="gb")
                    off = (ch % SLABCH) * (CHUNK // 16)
                    nc.gpsimd.dma_gather(
                        gb[:], win_ap_l(li, wq),
                        idx_slab[:, off:off + CHUNK // 16],
                        CHUNK, nidx_reg, F, single_packet=True,
                        queue_num=ch % NQUEUE)
                    gt0 = ch * TPC
                    any_part = any(tile_parts[gt0 + t] for t in range(TPC))
                    seg = None
                    if any_part:
                        seg = segpool.tile([128, TPC, 64], BF16, tag="seg")
                        nc.vector.tensor_tensor(
                            out=seg[:],
                            in0=iota64[:].rearrange("p (a b) -> p a b", a=TPC),
                            in1=slot_t[:, gt0:gt0 + TPC].unsqueeze(2)
                                .to_broadcast([128, TPC, 64]),
                            op=AOP.is_equal)
                    for t in range(TPC):
                        gt = gt0 + t
                        parts = tile_parts[gt]
                        for pi in range(len(parts)):
                            b = part_blk[gt, pi]
                            o, blo, W = band[gt, pi]
                            is_last = last_part.get((h, b)) == (gt, pi)
                            stopf = is_last and (h == 0 or li == 0)
                            bt, co = aggt[b]
                            nc.tensor.matmul(
                                out=bt[:, co + o:co + o + W],
                                lhsT=gb[:, t, :],
                                rhs=seg[:, t, blo:blo + W],
                                start=False, stop=stopf,
                                skip_group_check=True)
                        for b in fin_after.get((h, gt), ()):
                            if h == 0:
                                close_alpha(b)
                            else:
                                emit_epilogue(b)

                # ---- stats AllReduce chain (deferred into next layer) ----
                def emit_stats_chain(_stats=stats, _ctx=layer_ctx, _li=li,
                                     _last=last):
                    st_in = dram2.tile([128, 2], F32, tag="stin")
                    st_out = dram2.tile([128, 2], F32, tag="stout")
                    nc.sync.dma_start(out=st_in[:], in_=_stats[:])
                    nc.gpsimd.collective_compute(
                        "AllReduce", AOP.add, replica_groups=rg,
                        ins=[st_in[:]], outs=[st_out[:]])
                    stg = work.tile([128, 2], F32, tag="stg")
                    nc.sync.dma_start(out=stg[:], in_=st_out[:])
                    stg16 = work.tile([128, 2], BF16, tag="stg16")
                    nc.vector.tensor_copy(out=stg16[:], in_=stg[:])
                    ps_s = mmp.tile([1, 2], F32, tag="mm")
                    nc.tensor.matmul(out=ps_s[:], lhsT=ones_col[:], rhs=stg16[:],
                                     start=True, stop=True)
                    sc = work.tile([1, 4], F32, tag="sc")
                    nc.scalar.activation(out=sc[:, 0:2], in_=ps_s[:], func=AF.Copy,
                                         bias=0.0, scale=1.0 / (N_NODES * F))
                    nc.vector.tensor_tensor(out=sc[:, 2:3], in0=sc[:, 0:1],
                                            in1=sc[:, 0:1], op=AOP.mult)
                    nc.vector.tensor_tensor(out=sc[:, 2:3], in0=sc[:, 1:2],
                                            in1=sc[:, 2:3], op=AOP.subtract)
                    nc.vector.tensor_scalar(out=sc[:, 2:3], in0=sc[:, 2:3],
                                            scalar1=EPS, scalar2=None, op0=AOP.add)
                    nc.vector.reciprocal(out=sc[:, 3:4], in_=sc[:, 2:3])
                    nc.scalar.activation(out=sc[:, 3:4], in_=sc[:, 3:4],
                                         func=AF.Sqrt, bias=0.0, scale=1.0)
                    sc16 = work.tile([1, 4], BF16, tag="sc16")
                    nc.vector.tensor_copy(out=sc16[:], in_=sc[:])
                    ps_b = mmp.tile([128, 4], F32, tag="mm")
                    nc.tensor.matmul(out=ps_b[:], lhsT=ones_row1[:], rhs=sc16[:],
                                     start=True, stop=True)
                    musd = work.tile([128, 4], F32, tag="musd")
                    nc.vector.tensor_copy(out=musd[:], in_=ps_b[:])
                    _ctx["musd"] = musd
                    if not _last:
                        wmrow = work.tile([1, 128], BF16, tag="wmrow")
                        nc.vector.tensor_scalar(
                            out=wmrow[:],
                            in0=wbarnr[:, _li * F:(_li + 1) * F],
                            scalar1=musd[0:1, 0:1], scalar2=None,
                            op0=AOP.mult)
                        _ctx["wmrow"] = wmrow

                if last:
                    emit_stats_chain()
                else:
                    pending_chain = emit_stats_chain

            # ---------------- pool affine + MLP head ----------------
            pooledT = work.tile([128, NGRAPH], F32, tag="pooledT")
            nc.scalar.activation(out=pooledT[:], in_=pool_ps[:],
                                 func=AF.Copy, bias=0.0, scale=1.0)
            pl_in = dram2.tile([128, NGRAPH], F32, tag="plin")
            pl_out = dram2.tile([128, NGRAPH], F32, tag="plout")
            nc.sync.dma_start(out=pl_in[:], in_=pooledT[:])
            nc.gpsimd.collective_compute(
                "AllReduce", AOP.add, replica_groups=rg,
                ins=[pl_in[:]], outs=[pl_out[:]])
            pooled = work.tile([128, NGRAPH], F32, tag="pooled2")
            nc.sync.dma_start(out=pooled[:], in_=pl_out[:])
            nc.vector.tensor_tensor(out=pooled[:], in0=pooled[:],
                                    in1=invcntr[:], op=AOP.mult)
            mrs = work.tile([128, 1], F32, tag="mrs")
            nc.vector.tensor_tensor(out=mrs[:], in0=layer_ctx["musd"][:, 0:1],
                                    in1=layer_ctx["musd"][:, 3:4], op=AOP.mult)
            nc.vector.tensor_scalar(out=pooled[:], in0=pooled[:],
                                    scalar1=layer_ctx["musd"][:, 3:4],
                                    scalar2=mrs[:],
                                    op0=AOP.mult, op1=AOP.subtract)
            pooled16 = work.tile([128, NGRAPH], BF16, tag="pooled16")
            nc.vector.tensor_copy(out=pooled16[:], in_=pooled[:])

            ps_g = mmp.tile([128, NGRAPH], F32, tag="mm")
            nc.tensor.matmul(out=ps_g[:], lhsT=mlpW1[:], rhs=pooled16[:],
                             start=True, stop=True)
            gT = work.tile([128, NGRAPH], BF16, tag="gT")
            nc.scalar.activation(out=gT[:], in_=ps_g[:], func=AF.Relu,
                                 bias=mlpb1[:], scale=1.0)
            for halfi in range(2):
                ps_sc = mmp.tile([128, NCLS], F32, tag="mm")
                nc.tensor.matmul(out=ps_sc[:],
                                 lhsT=gT[:, halfi * 128:(halfi + 1) * 128],
                                 rhs=mlpW2[:], start=True, stop=True)
                scr = work.tile([128, NCLS], F32, tag="scr")
                nc.vector.tensor_tensor(out=scr[:], in0=ps_sc[:],
                                        in1=mlpb2r[:], op=AOP.add)
                mx = work.tile([128, 1], F32, tag="mx")
                nc.vector.tensor_reduce(out=mx[:], in_=scr[:],
                                        axis=mybir.AxisListType.X,
                                        op=AOP.max)
                nc.vector.tensor_scalar(out=scr[:], in0=scr[:], scalar1=mx[:],
                                        scalar2=None, op0=AOP.subtract)
                ex = work.tile([128, NCLS], F32, tag="ex")
                sm = work.tile([128, 1], F32, tag="sm")
                nc.scalar.activation(out=ex[:], in_=scr[:], func=AF.Exp,
                                     bias=0.0, scale=1.0, accum_out=sm[:])
                ls = work.tile([128, 1], F32, tag="ls")
                nc.scalar.activation(out=ls[:], in_=sm[:], func=AF.Ln,
                                     bias=0.0, scale=1.0)
                nc.vector.tensor_scalar(out=scr[:], in0=scr[:], scalar1=ls[:],
                                        scalar2=None, op0=AOP.subtract)
                nc.sync.dma_start(
                    out=out_ext[halfi * 128:(halfi + 1) * 128, :],
                    in_=scr[:])

    nc.compile()
    return nc


def _wrap_cols(vec, fill):
    padded = np.full(NBLK * 128, fill, np.float32)
    padded[:NSH] = vec
    return np.ascontiguousarray(padded.reshape(NBLK, 128).T)


def _prepare(inputs):
    x = np.asarray(inputs["x"], dtype=np.float32)
    edge_index = np.asarray(inputs["edge_index"])
    batch = np.asarray(inputs["batch"], dtype=np.int64)

    dinv, sig, idxw, slotw, meta = _host_preprocess(edge_index)

    cnt = np.bincount(batch, minlength=NGRAPH).astype(np.float64)
    invcnt = (1.0 / np.maximum(cnt, 1.0)).astype(np.float32)
    iota64 = np.tile(np.arange(64, dtype=np.float32), (128, TPC))
    iota256 = np.broadcast_to(np.arange(256, dtype=np.float32), (128, 256))

    lin1_W = np.asarray(inputs["lin1_W"], np.float32)
    lin1_b = np.asarray(inputs["lin1_b"], np.float32)
    conv_W = np.asarray(inputs["conv_W"], np.float32)
    conv_b = np.asarray(inputs["conv_b"], np.float32)
    mlp_W1 = np.asarray(inputs["mlp_W1"], np.float32)
    mlp_b1 = np.asarray(inputs["mlp_b1"], np.float32)
    mlp_W2 = np.asarray(inputs["mlp_W2"], np.float32)
    mlp_b2 = np.asarray(inputs["mlp_b2"], np.float32)

    convW_cat = np.concatenate([conv_W[l] for l in range(LAYERS)], axis=1)
    wbarnr = -np.concatenate(
        [conv_W[l].sum(axis=0) for l in range(1, LAYERS)])[None, :]

    in_maps = []
    for c in range(NCORES):
        lo, hi = c * NSH, (c + 1) * NSH
        xT = np.zeros((F, NBLK * 128), np.float32)
        xT[:, :NSH] = x[lo:hi].T
        dinv_pad = np.zeros(NBLK * 128, np.float32)
        dinv_pad[:NSH] = dinv[lo:hi]
        sig_pad = np.zeros(NBLK * 128, np.float32)
        sig_pad[:NSH] = sig[lo:hi]
        in_maps.append({
            "xT": xT.astype(BF),
            "idx": idxw[c],
            "slot": slotw[c],
            "dinvrep": np.ascontiguousarray(
                np.broadcast_to(dinv_pad, (128, NBLK * 128))).astype(BF),
            "sigrow": np.ascontiguousarray(sig_pad[None, :]).astype(BF),
            "dinvw": _wrap_cols(dinv[lo:hi], 0.0),
            "pslot": _wrap_cols(batch[lo:hi].astype(np.float32),
                                300.0).astype(BF),
            "iota64": iota64.astype(BF),
            "iota256": iota256.astype(BF),
            "lin1W": lin1_W.astype(BF),
            "lin1b": np.ascontiguousarray(lin1_b.reshape(F, 1)),
            "convW": convW_cat.astype(BF),
            "convb": np.ascontiguousarray(conv_b.T),
            "wbarnr": np.ascontiguousarray(wbarnr.astype(np.float32)),
            "mlpW1": mlp_W1.astype(BF),
            "mlpb1": np.ascontiguousarray(mlp_b1.reshape(F, 1)),
            "mlpW2": mlp_W2.astype(BF),
            "mlpb2r": np.ascontiguousarray(
                np.broadcast_to(mlp_b2, (128, NCLS)).astype(np.float32)),
            "invcntr": np.ascontiguousarray(
                np.broadcast_to(invcnt, (128, NGRAPH))),
        })
    return meta, in_maps


_CACHED = {}


def kernel_run(inputs, trace=False):
    meta, in_maps = _prepare(inputs)
    key = meta["TT"]
    if key not in _CACHED:
        _CACHED[key] = _build_program(meta)
    nc = _CACHED[key]
    res = run_bass_kernel_spmd(nc, in_maps, core_ids=list(range(NCORES)),
                               trace=trace)
    out = np.asarray(res.results[0]["out"], dtype=np.float32)
    return out, res.exec_time_ns


def kernel(**inputs):
    out, _ = kernel_run(inputs, trace=False)
    return out
